# revision 9
# baseline (speedup 1.0000x reference)
"""Trainium2 Bass kernel for the MoE-routing Actor network (8 NeuronCores).

Data-parallel over batch (512 rows/core). fc2 (the dominant 8192x8192 GEMM)
runs in fp8-e4m3 DoubleRow mode (256-deep contraction per instruction, 2x
bf16 PE throughput) with fc2_W pre-cast and pre-tiled on the host into a
[group, kdpair, part, 2, col] fp8 layout (64MB streamed instead of 256MB
fp32). h1 is quantized to fp8 with a x16 scale folded into the LayerNorm1
ReLU eviction; W2 carries a x128 scale; the PSUM eviction descales by
1/2048. Everything else (fc1, gate/top-k, LayerNorms, mixture, heads) is
bf16/fp32 exactly as the dense baseline:
  - fc2 output h2 is batch-major [512, 8192] bf16 (16 groups of 512
    columns), so LayerNorm2 uses per-partition tensor_scalar ops, the
    expert-score multiply uses a stride-0 expanded AP, and the
    group-of-16 mean is a free-axis reduce.
  - heads via PE transpose of mixed + one combined matmul.
"""

import numpy as np
import ml_dtypes

import concourse.bass as bass
import concourse.bacc as bacc
import concourse.mybir as mybir
import concourse.tile as tile
from concourse.bass_utils import run_bass_kernel_spmd

F32 = mybir.dt.float32
BF16 = mybir.dt.bfloat16
FP8 = mybir.dt.float8e4
AF = mybir.ActivationFunctionType
ALU = mybir.AluOpType
AX = mybir.AxisListType
DR = mybir.MatmulPerfMode.DoubleRow

N_CORES = 8
B, OBS, ACT_DIM, H, M, TOPK = 4096, 256, 32, 512, 16, 4
D = H * M          # 8192 trunk width
BL = B // N_CORES  # 512 local batch rows
P = 128
NKT = D // P       # 64 k tiles over trunk width
NKD = NKT // 2     # 32 DoubleRow k-pair tiles
NBT = BL // P      # 4 batch tiles of the local shard
NCH = 16           # fc2 512-column groups
GRP = P // M       # 8 expert groups per 128 columns
LN_EPS = 1e-5
LOG_STD_MAX, LOG_STD_MIN = 2.0, -5.0
SX = 16.0          # h1 fp8 scale
SW = 128.0         # fc2_W fp8 scale
DESCALE = 1.0 / (SX * SW)

DEBUG_TAPS = False


def build_kernel(b2_trivial=True):
    nc = bacc.Bacc(None, target_bir_lowering=False, num_devices=N_CORES)

    x_ext = nc.declare_dram_parameter("x", [BL, OBS], F32, isOutput=False)
    gw_ext = nc.declare_dram_parameter("gate_W", [OBS, M], F32, isOutput=False)
    gb_ext = nc.declare_dram_parameter("gate_b", [M], F32, isOutput=False)
    w1_ext = nc.declare_dram_parameter("fc1_W", [OBS, D], F32, isOutput=False)
    b1_ext = nc.declare_dram_parameter("fc1_b", [D], F32, isOutput=False)
    n1s_ext = nc.declare_dram_parameter("norm1_scale", [D], F32, isOutput=False)
    n1b_ext = nc.declare_dram_parameter("norm1_bias", [D], F32, isOutput=False)
    w28_ext = nc.declare_dram_parameter("fc2_W8", [NCH * NKD * P, 2 * BL], FP8,
                                        isOutput=False)
    b2_ext = nc.declare_dram_parameter("fc2_b", [D], F32, isOutput=False)
    mw_ext = nc.declare_dram_parameter("mean_W", [H, ACT_DIM], F32, isOutput=False)
    mb_ext = nc.declare_dram_parameter("mean_b", [ACT_DIM], F32, isOutput=False)
    lw_ext = nc.declare_dram_parameter("logstd_W", [H, ACT_DIM], F32, isOutput=False)
    lb_ext = nc.declare_dram_parameter("logstd_b", [ACT_DIM], F32, isOutput=False)
    out_ext = nc.declare_dram_parameter("out", [BL, 2 * ACT_DIM], F32, isOutput=True)
    taps = {}
    if DEBUG_TAPS:
        taps["scores"] = nc.declare_dram_parameter("tap_scores", [BL, M], F32, isOutput=True)
        taps["h1"] = nc.declare_dram_parameter("tap_h1", [P, BL], F32, isOutput=True)
        taps["mixed"] = nc.declare_dram_parameter("tap_mixed", [BL, H], F32, isOutput=True)

    ident_dram = nc.inline_tensor(np.eye(P, dtype=np.float32), name="ident")
    ones_row_dram = nc.inline_tensor(np.ones((1, P), np.float32), name="ones_row")

    with tile.TileContext(nc) as tc:
        with (
            tc.tile_pool(name="cst", bufs=1) as cst,
            tc.tile_pool(name="pp", bufs=8, space="PSUM") as pp,
        ):
            # ---------------- constants / small parameters -----------------
            ident = cst.tile([P, P], F32)
            nc.sync.dma_start(ident[:], ident_dram[:])
            identb = cst.tile([P, P], BF16)
            nc.vector.tensor_copy(identb[:], ident[:])
            ones_row_f = cst.tile([1, P], F32)
            nc.sync.dma_start(ones_row_f[:], ones_row_dram[:])
            ones_row_b = cst.tile([1, P], BF16)
            nc.vector.tensor_copy(ones_row_b[:], ones_row_f[:])
            eps_t = cst.tile([1, 1], F32)
            nc.any.memset(eps_t[:], LN_EPS)
            ones_col_b = cst.tile([P, 1], BF16)
            nc.any.memset(ones_col_b[:], 1.0)
            eps_col = cst.tile([P, 1], F32)
            nc.any.memset(eps_col[:], LN_EPS)

            def load_feat_vec(ext, n, nm):
                """[n*P] DRAM vector -> [P, n] SBUF tile (feature-on-partition)."""
                staged = cst.tile([NKT, P], F32, tag="bstage", bufs=2, name=f"{nm}_st")
                nc.sync.dma_start(staged[0:n, :], ext.ap().rearrange("(a b) -> a b", b=P))
                dst = cst.tile([P, n], F32, name=nm)
                tp_ = pp.tile([P, NKT], F32, tag="ps", name=f"{nm}_tp")
                nc.tensor.transpose(tp_[0:P, 0:n], staged[0:n, :], ident[0:n, 0:n])
                nc.scalar.activation(dst[:], tp_[0:P, 0:n], AF.Copy)
                return dst

            fc1b = load_feat_vec(b1_ext, NKT, "fc1b")
            n1s = load_feat_vec(n1s_ext, NKT, "n1s")
            n1b = load_feat_vec(n1b_ext, NKT, "n1b")
            # fold the fp8 x-scale into the LN1 affine params
            n1sS = cst.tile([P, NKT], F32)
            nc.vector.tensor_scalar_mul(n1sS[:], n1s[:], SX)
            n1bS = cst.tile([P, NKT], F32)
            nc.vector.tensor_scalar_mul(n1bS[:], n1b[:], SX)

            gwf = cst.tile([P, 2 * M], F32)
            for kt in range(2):
                nc.sync.dma_start(gwf[:, kt * M:(kt + 1) * M],
                                  gw_ext[kt * P:(kt + 1) * P, :])
            gbf = cst.tile([1, M], F32)
            nc.sync.dma_start(gbf[:], gb_ext.ap().rearrange("(a b) -> a b", a=1))

            # head weights [512, 64] bf16 (mean | logstd), 4 k-tiles
            hwt_f = cst.tile([P, 4 * 2 * ACT_DIM], F32)
            for ht in range(4):
                nc.sync.dma_start(hwt_f[:, ht * 2 * ACT_DIM: ht * 2 * ACT_DIM + ACT_DIM],
                                  mw_ext[ht * P:(ht + 1) * P, :])
                nc.sync.dma_start(hwt_f[:, ht * 2 * ACT_DIM + ACT_DIM:(ht + 1) * 2 * ACT_DIM],
                                  lw_ext[ht * P:(ht + 1) * P, :])
            hwt = cst.tile([P, 4 * 2 * ACT_DIM], BF16)
            nc.vector.tensor_copy(hwt[:], hwt_f[:])
            hb_f = cst.tile([1, 2 * ACT_DIM], F32)
            nc.sync.dma_start(hb_f[:, 0:ACT_DIM], mb_ext.ap().rearrange("(a b) -> a b", a=1))
            nc.sync.dma_start(hb_f[:, ACT_DIM:2 * ACT_DIM],
                              lb_ext.ap().rearrange("(a b) -> a b", a=1))
            hbb = cst.tile([1, 2 * ACT_DIM], BF16)
            nc.vector.tensor_copy(hbb[:], hb_f[:])

            xT = cst.tile([P, 2 * BL], BF16)    # x^T k-tiles side by side
            zws = cst.tile([P, NKT * BL], BF16)  # fc1 pre-norm workspace
            h1n8 = cst.tile([P, NKT * BL], FP8)  # normalized h1, fp8 x16
            scb = cst.tile([P, NBT * M], BF16)   # top-k scores per batch tile
            sxp = cst.tile([P, 2 * NBT * NCH], F32)  # per-chunk sum/sumsq partials

            # w2 fp8 stream pool opened before p1 so g=0 chunks preload
            # during phase 1 (p2s outlives p1; LIFO respected)
            _p2s_cm = tc.tile_pool(name="p2s", bufs=1)
            p2s = _p2s_cm.__enter__()

            def w2_load(g, kd):
                w2c = p2s.tile([P, 2 * BL], FP8, tag="w2c", bufs=16,
                               name=f"w2c{g}_{kd}")
                base = (g * NKD + kd) * P
                nc.sync.dma_start(w2c[:], w28_ext[base:base + P, :])
                return w2c

            w2pre = {}

            # ================= phase 0 + 1 (pool p1) ========================
            with tc.tile_pool(name="p1", bufs=1) as p1:
                xTf = p1.tile([P, 2 * BL], F32, tag="xTf", bufs=1, name="xTf")
                for bt in range(NBT):
                    xl = p1.tile([P, OBS], F32, tag="xload", bufs=2, name=f"xl{bt}")
                    nc.sync.dma_start(xl[:], x_ext[bt * P:(bt + 1) * P, :])
                    for kt in range(2):
                        tp = pp.tile([P, P], F32, tag="ps", name=f"xtp{bt}_{kt}")
                        nc.tensor.transpose(tp[:], xl[:, kt * P:(kt + 1) * P], ident[:])
                        nc.scalar.activation(
                            xTf[:, kt * BL + bt * P: kt * BL + (bt + 1) * P],
                            tp[:], AF.Copy)
                        nc.vector.tensor_copy(
                            xT[:, kt * BL + bt * P: kt * BL + (bt + 1) * P], tp[:])

                # ---- fc1 (bf16) + LN1 stats ----
                w1b = []
                for kt in range(2):
                    w1t = p1.tile([P, D], BF16, tag=f"w1b{kt}", bufs=1, name=f"w1b{kt}")
                    for h in range(4):
                        w1f = p1.tile([P, D // 4], F32, tag="w1f", bufs=2,
                                      name=f"w1f{kt}_{h}")
                        nc.sync.dma_start(
                            w1f[:], w1_ext[kt * P:(kt + 1) * P,
                                           h * (D // 4):(h + 1) * (D // 4)])
                        nc.vector.tensor_copy(w1t[:, h * (D // 4):(h + 1) * (D // 4)],
                                              w1f[:])
                    w1b.append(w1t)

                # ---- gate + softmax + top-4 (fp32) ----
                for bt in range(NBT):
                    gp = pp.tile([P, M], F32, tag="ps", name=f"gp{bt}")
                    for kt in range(2):
                        nc.tensor.matmul(
                            gp[:], xTf[:, kt * BL + bt * P: kt * BL + (bt + 1) * P],
                            gwf[:, kt * M:(kt + 1) * M], start=(kt == 0), stop=False)
                    nc.tensor.matmul(gp[:], ones_row_f[:], gbf[:], start=False, stop=True)

                    def g1(nm):
                        return p1.tile([P, 1], F32, tag="gs1", bufs=6, name=f"{nm}{bt}")

                    def g16(nm):
                        return p1.tile([P, M], F32, tag="gs16", bufs=6, name=f"{nm}{bt}")

                    gmax = g1("gmax")
                    nc.vector.tensor_reduce(gmax[:], gp[:], AX.X, ALU.max)
                    ngmax = g1("ngmax")
                    nc.vector.tensor_scalar_mul(ngmax[:], gmax[:], -1.0)
                    ge = g16("ge")
                    nc.scalar.activation(ge[:], gp[:], AF.Exp, bias=ngmax[:])
                    gsum = g1("gsum")
                    nc.vector.reduce_sum(gsum[:], ge[:], axis=AX.X)
                    grec = g1("grec")
                    nc.vector.reciprocal(grec[:], gsum[:])
                    s0 = g16("s0")
                    nc.vector.tensor_scalar_mul(s0[:], ge[:], grec[:])
                    mt4 = p1.tile([P, TOPK], F32, tag="gs4", bufs=2, name=f"mt4{bt}")
                    w = s0
                    for t in range(TOPK):
                        nc.vector.tensor_reduce(mt4[:, t:t + 1], w[:], AX.X, ALU.max)
                        if t < TOPK - 1:
                            msk = g16(f"msk{t}_")
                            nc.vector.tensor_scalar(msk[:], w[:], mt4[:, t:t + 1], None,
                                                    op0=ALU.is_ge)
                            w2_ = g16(f"w{t}_")
                            nc.vector.tensor_tensor(w2_[:], w[:], msk[:], op=ALU.subtract)
                            w = w2_
                    tsum = g1("tsum")
                    nc.vector.reduce_sum(tsum[:], mt4[:], axis=AX.X)
                    trec = g1("trec")
                    nc.vector.reciprocal(trec[:], tsum[:])
                    keep = g16("keep")
                    nc.vector.tensor_scalar(keep[:], s0[:], mt4[:, TOPK - 1:TOPK], None,
                                            op0=ALU.is_ge)
                    sn = g16("sn")
                    nc.vector.tensor_scalar_mul(sn[:], s0[:], trec[:])
                    sc = g16("sc")
                    nc.vector.tensor_tensor(sc[:], sn[:], keep[:], op=ALU.mult)
                    nc.vector.tensor_copy(scb[:, bt * M:(bt + 1) * M], sc[:])
                    if DEBUG_TAPS:
                        nc.sync.dma_start(taps["scores"][bt * P:(bt + 1) * P, :], sc[:])

                st1x = pp.tile([1, BL], F32, tag="ps", name="st1x")
                st1q = pp.tile([1, BL], F32, tag="ps", name="st1q")
                for nt in range(NKT):
                    ps1 = pp.tile([P, BL], F32, tag="ps", name=f"ps1_{nt}")
                    for kt in range(2):
                        nc.tensor.matmul(ps1[:], w1b[kt][:, nt * P:(nt + 1) * P],
                                         xT[:, kt * BL:(kt + 1) * BL],
                                         start=(kt == 0), stop=(kt == 1))
                    h1r = zws[:, nt * BL:(nt + 1) * BL]
                    nc.scalar.activation(h1r, ps1[:], AF.Identity,
                                         bias=fc1b[:, nt:nt + 1])
                    sq = p1.tile([P, BL], BF16, tag="sq1", bufs=3, name=f"sq1_{nt}")
                    nc.vector.tensor_tensor(sq[:], h1r, h1r, op=ALU.mult)
                    nc.tensor.matmul(st1x[:], ones_col_b[:], h1r,
                                     start=(nt == 0), stop=(nt == NKT - 1))
                    nc.tensor.matmul(st1q[:], ones_col_b[:], sq[:],
                                     start=(nt == 0), stop=(nt == NKT - 1))

                # LN1 stats -> broadcast tiles; normalize into h1n8
                sx1 = p1.tile([1, BL], F32, tag="ln1v", bufs=6, name="sx1")
                nc.vector.tensor_copy(sx1[:], st1x[:])
                sq1v = p1.tile([1, BL], F32, tag="ln1v", bufs=6, name="sq1v")
                nc.vector.tensor_copy(sq1v[:], st1q[:])

                def v1(nm):
                    return p1.tile([1, BL], F32, tag="ln1v", bufs=6, name=nm)
                mu = v1("muL1")
                nc.vector.tensor_scalar_mul(mu[:], sx1[:], 1.0 / D)
                vb = p1.tile([1, 2 * BL], BF16, tag="ln1vb", bufs=1, name="vbL1")
                nc.vector.tensor_copy(vb[:, BL:2 * BL], mu[:])
                mu2 = v1("mu2L1")
                nc.scalar.activation(mu2[:], mu[:], AF.Square)
                e2 = v1("e2L1")
                nc.vector.tensor_scalar_mul(e2[:], sq1v[:], 1.0 / D)
                var = v1("varL1")
                nc.vector.tensor_tensor(var[:], e2[:], mu2[:], op=ALU.subtract)
                sd = v1("sdL1")
                nc.scalar.activation(sd[:], var[:], AF.Sqrt, bias=eps_t[:])
                inv = v1("invL1")
                nc.vector.reciprocal(inv[:], sd[:])
                nc.vector.tensor_copy(vb[:, 0:BL], inv[:])
                invB_ps = pp.tile([P, BL], F32, tag="ps", name="invBpsL1")
                nc.tensor.matmul(invB_ps[:], ones_row_b[:], vb[:, 0:BL],
                                 start=True, stop=True)
                invB = p1.tile([P, BL], BF16, tag="ln1bc", bufs=2, name="invBL1")
                nc.scalar.activation(invB[:], invB_ps[:], AF.Copy)
                muB_ps = pp.tile([P, BL], F32, tag="ps", name="muBpsL1")
                nc.tensor.matmul(muB_ps[:], ones_row_b[:], vb[:, BL:2 * BL],
                                 start=True, stop=True)
                muB = p1.tile([P, BL], BF16, tag="ln1bc", bufs=2, name="muBL1")
                nc.scalar.activation(muB[:], muB_ps[:], AF.Copy)

                for nt in range(NKT):
                    u = p1.tile([P, BL], BF16, tag="n1u", bufs=3, name=f"u{nt}")
                    nc.vector.tensor_tensor(u[:], zws[:, nt * BL:(nt + 1) * BL],
                                            muB[:], op=ALU.subtract)
                    v_ = p1.tile([P, BL], BF16, tag="n1v", bufs=3, name=f"v{nt}")
                    nc.vector.tensor_tensor(v_[:], u[:], invB[:], op=ALU.mult)
                    nc.scalar.activation(h1n8[:, nt * BL:(nt + 1) * BL], v_[:], AF.Relu,
                                         scale=n1sS[:, nt:nt + 1], bias=n1bS[:, nt:nt + 1])
                    if nt % 2 == 0 and nt // 2 < NKD // 2:
                        w2pre[(0, nt // 2)] = w2_load(0, nt // 2)
                    if DEBUG_TAPS and nt == 3:
                        hf = p1.tile([P, BL], F32, tag="tapf", bufs=1, name="hf")
                        nc.vector.tensor_copy(hf[:], h1n8[:, nt * BL:(nt + 1) * BL])
                        nc.sync.dma_start(taps["h1"][:], hf[:])

            # ================= phase 2: fc2 fp8 DR (batch-major out) ========
            with tc.tile_pool(name="p2", bufs=1) as p2:
                h2 = [p2.tile([P, NCH * BL], BF16, name=f"h2_{bt}")
                      for bt in range(NBT)]
                h18v = h1n8[:].rearrange("p (k b) -> p k b", b=BL)
                if not b2_trivial:
                    fc2b = p2.tile([1, D], BF16, name="fc2b")
                    for h in range(4):
                        f2s = p2.tile([1, D // 4], F32, tag="f2s", bufs=2, name=f"f2s{h}")
                        nc.sync.dma_start(
                            f2s[:], b2_ext.ap().rearrange("(a b) -> a b", a=1)
                            [:, h * (D // 4):(h + 1) * (D // 4)])
                        # bias is added inside the scaled-PSUM domain
                        nc.vector.tensor_scalar_mul(
                            fc2b[:, h * (D // 4):(h + 1) * (D // 4)], f2s[:], SX * SW)
                for g in range(NCH):
                    ps2 = [pp.tile([P, BL], F32, tag="ps", name=f"ps2_{g}_{bt}")
                           for bt in range(NBT)]
                    for kd in range(NKD):
                        if g == 0 and kd < NKD // 2:
                            w2c = w2pre.pop((0, kd))
                        else:
                            w2c = w2_load(g, kd)
                        w2cv = w2c[:].rearrange("p (two b) -> p two b", two=2)
                        for bt in range(NBT):
                            nc.tensor.matmul(
                                ps2[bt][:],
                                h18v[:, 2 * kd:2 * kd + 2, bt * P:(bt + 1) * P],
                                w2cv,
                                start=(kd == 0),
                                stop=(b2_trivial and kd == NKD - 1),
                                perf_mode=DR)
                    if not b2_trivial:
                        for bt in range(NBT):
                            nc.tensor.matmul(
                                ps2[bt][:], ones_row_b[:],
                                fc2b[:, g * BL:(g + 1) * BL],
                                start=False, stop=True)
                    # evict to bf16 h2 (descaled); row stats ride accum_out
                    for bt in range(NBT):
                        dst = h2[bt][:, g * BL:(g + 1) * BL]
                        nc.scalar.activation(
                            dst, ps2[bt][:], AF.Copy, scale=DESCALE,
                            accum_out=sxp[:, (bt * NCH + g) * 2:
                                          (bt * NCH + g) * 2 + 1])
                        sqc = p2.tile([P, BL], BF16, tag="sq2", bufs=2,
                                      name=f"sq2_{g}_{bt}")
                        nc.scalar.activation(
                            sqc[:], dst, AF.Square,
                            accum_out=sxp[:, (bt * NCH + g) * 2 + 1:
                                          (bt * NCH + g) * 2 + 2])

                # ---- LN2 finalize + mixture + heads, per batch tile ----
                mixed_tiles = []

                def emit_heads(bt):
                    mixed = mixed_tiles[bt]
                    mixb = p2.tile([P, H], BF16, tag="mixb", bufs=2, name=f"mixb{bt}")
                    nc.scalar.mul(mixb[:], mixed[:], 1.0 / M)
                    mts = []
                    for ht in range(4):
                        mtp = pp.tile([P, P], BF16, tag="ps", name=f"mtp{bt}_{ht}")
                        nc.tensor.transpose(mtp[:], mixb[:, ht * P:(ht + 1) * P],
                                            identb[:])
                        mt_ = p2.tile([P, P], BF16, tag="mixT", bufs=5,
                                      name=f"mt{bt}_{ht}")
                        nc.scalar.activation(mt_[:], mtp[:], AF.Copy)
                        mts.append(mt_)
                    hps = pp.tile([P, 2 * ACT_DIM], F32, tag="ps", name=f"hps{bt}")
                    for ht in range(4):
                        nc.tensor.matmul(hps[:], mts[ht][:],
                                         hwt[:, ht * 2 * ACT_DIM:(ht + 1) * 2 * ACT_DIM],
                                         start=(ht == 0), stop=False)
                    nc.tensor.matmul(hps[:], ones_row_b[:], hbb[:],
                                     start=False, stop=True)
                    ho = p2.tile([P, 2 * ACT_DIM], F32, tag="ho", bufs=2, name=f"ho{bt}")
                    nc.vector.tensor_copy(ho[:, 0:ACT_DIM], hps[:, 0:ACT_DIM])
                    th = p2.tile([P, ACT_DIM], F32, tag="th", bufs=2, name=f"th{bt}")
                    nc.scalar.activation(th[:], hps[:, ACT_DIM:2 * ACT_DIM], AF.Tanh)
                    nc.vector.tensor_scalar(
                        ho[:, ACT_DIM:2 * ACT_DIM], th[:],
                        0.5 * (LOG_STD_MAX - LOG_STD_MIN),
                        LOG_STD_MIN + 0.5 * (LOG_STD_MAX - LOG_STD_MIN),
                        op0=ALU.mult, op1=ALU.add)
                    nc.sync.dma_start(out_ext[bt * P:(bt + 1) * P, :], ho[:])

                inv_t, nmi_t = [], []
                for bt in range(NBT):
                    def l2(nm):
                        return p2.tile([P, 1], F32, tag="l2s", bufs=44,
                                       name=f"{nm}_{bt}")
                    sx = l2("sx2")
                    nc.vector.tensor_reduce(
                        sx[:], sxp[:, bt * 2 * NCH:(bt + 1) * 2 * NCH].rearrange(
                            "p (c two) -> p c two", two=2)[:, :, 0:1], AX.XY, ALU.add)
                    sq_ = l2("sq2v")
                    nc.vector.tensor_reduce(
                        sq_[:], sxp[:, bt * 2 * NCH:(bt + 1) * 2 * NCH].rearrange(
                            "p (c two) -> p c two", two=2)[:, :, 1:2], AX.XY, ALU.add)
                    mu = l2("mu2")
                    nc.vector.tensor_scalar_mul(mu[:], sx[:], 1.0 / D)
                    mu2 = l2("mu22")
                    nc.scalar.activation(mu2[:], mu[:], AF.Square)
                    e2 = l2("e22")
                    nc.vector.tensor_scalar_mul(e2[:], sq_[:], 1.0 / D)
                    var = l2("var2")
                    nc.vector.tensor_tensor(var[:], e2[:], mu2[:], op=ALU.subtract)
                    sd = l2("sd2")
                    nc.scalar.activation(sd[:], var[:], AF.Sqrt, bias=eps_col[:])
                    inv = l2("inv2")
                    nc.vector.reciprocal(inv[:], sd[:])
                    nmi = l2("nmi2")
                    nc.vector.tensor_scalar(nmi[:], mu[:], inv[:], -1.0,
                                            op0=ALU.mult, op1=ALU.mult)
                    inv_t.append(inv)
                    nmi_t.append(nmi)

                for bt in range(NBT):
                    inv, nmi = inv_t[bt], nmi_t[bt]
                    mixed = p2.tile([P, H], F32, tag="mixed", bufs=3,
                                    name=f"mixed_{bt}")
                    QL = 2 * BL  # process 2 chunks (1024 cols) per op
                    for q in range(NCH // 2):
                        chunk = h2[bt][:, q * QL:(q + 1) * QL]
                        t_ = p2.tile([P, QL], BF16, tag="n2t", bufs=2,
                                     name=f"t2_{bt}_{q}")
                        nc.scalar.activation(t_[:], chunk, AF.Relu,
                                             scale=inv[:], bias=nmi[:])
                        pr = p2.tile([P, QL], BF16, tag="n2p", bufs=2,
                                     name=f"pr_{bt}_{q}")
                        scb_bc = scb[:, bt * M:(bt + 1) * M].rearrange(
                            "p (o m) -> p o m", o=1).to_broadcast((P, QL // M, M))
                        veng = nc.vector
                        veng.tensor_tensor(
                            pr[:].rearrange("p (g m) -> p g m", m=M),
                            t_[:].rearrange("p (g m) -> p g m", m=M),
                            scb_bc, op=ALU.mult)
                        nc.vector.tensor_reduce(
                            mixed[:, q * (QL // M):(q + 1) * (QL // M)],
                            pr[:].rearrange("p (g m) -> p g m", m=M), AX.X, ALU.add)
                    if DEBUG_TAPS:
                        nc.sync.dma_start(taps["mixed"][bt * P:(bt + 1) * P, :],
                                          mixed[:])
                    mixed_tiles.append(mixed)
                    if bt > 0:
                        emit_heads(bt - 1)
                    if bt == NBT - 1:
                        emit_heads(bt)

            _p2s_cm.__exit__(None, None, None)

    nc.compile()
    return nc


_NC_CACHE = {}


def _get_nc(b2_trivial=True):
    if b2_trivial not in _NC_CACHE:
        _NC_CACHE[b2_trivial] = build_kernel(b2_trivial=b2_trivial)
    return _NC_CACHE[b2_trivial]


def make_in_maps(inputs):
    def f32c(a):
        return np.ascontiguousarray(np.asarray(a, np.float32))

    x = f32c(inputs["x"])
    shared = {k: f32c(inputs[k]) for k in (
        "gate_W", "gate_b", "fc1_W", "fc1_b", "norm1_scale", "norm1_bias",
        "fc2_b", "mean_W", "mean_b", "logstd_W", "logstd_b")}
    w2 = np.asarray(inputs["fc2_W"], np.float32)
    w2q = np.clip(w2 * SW, -240.0, 240.0).astype(ml_dtypes.float8_e4m3)
    # [row=(kd,i,p), col=(g,c)] -> [g, kd, p, i, c]
    w2dr = np.ascontiguousarray(
        w2q.reshape(NKD, 2, P, NCH, BL).transpose(3, 0, 2, 1, 4)
        .reshape(NCH * NKD * P, 2 * BL))
    shared["fc2_W8"] = w2dr
    in_maps = []
    for i in range(N_CORES):
        m = dict(shared)
        m["x"] = np.ascontiguousarray(x[i * BL:(i + 1) * BL])
        in_maps.append(m)
    return in_maps


def assemble(res):
    out = np.concatenate([res.results[i]["out"] for i in range(N_CORES)], axis=0)
    return (np.ascontiguousarray(out[:, :ACT_DIM]),
            np.ascontiguousarray(out[:, ACT_DIM:]))


def kernel(**inputs):
    topk = int(inputs.get("topk", TOPK))
    assert topk == TOPK, f"kernel compiled for topk={TOPK}, got {topk}"
    b2_triv = not np.any(np.asarray(inputs["fc2_b"]))
    n2_triv = (np.all(np.asarray(inputs["norm2_scale"]) == 1.0)
               and not np.any(np.asarray(inputs["norm2_bias"])))
    assert n2_triv, "general norm2 scale/bias path not implemented"
    nc = _get_nc(b2_trivial=b2_triv)
    in_maps = make_in_maps(inputs)
    res = run_bass_kernel_spmd(nc, in_maps, core_ids=list(range(N_CORES)))
    mean, log_std = assemble(res)
    return mean, log_std
